# revision 13
# baseline (speedup 1.0000x reference)
"""DeeperHNN hypergraph message passing kernel for 8 Trainium2 NeuronCores.

Strategy (sharding_hint): nodes (and incidence entries, partitioned by vertex)
are sharded across 8 cores; hyperedge aggregates are computed as per-core
partials and AllReduced (replicated) since E << N; weight matrices replicated.

v2 changes vs baseline:
  - dma_gather issued as prepare_only + trigger_dma on 4 rotating SWDGE
    queues: the Pool engine only pays descriptor generation; drains overlap
    with PE/vector/scalar work and with each other.
  - One gather per ~40-slot group (block-aligned) instead of 8-slot chunks.
  - All dense math in bf16 (weights, activations, gathered values); PSUM/LN
    stats stay f32.
  - tt (pre-conv activations) stored row-major bf16; the theta matmul reads
    it through HWDGE DMA-transpose instead of PE transposes + an hT tensor.
  - LayerNorm gain/bias folded into the next layer's theta/lin weights when
    lnB == 0 and lnG > 0 (always true for this model); the LN tail is then a
    single fused scalar activation relu(x*rinv - mu*rinv).
  - h/tt/T/Ye DRAM traffic batched into multi-block DMAs.
"""

import math

import numpy as np

import concourse.bacc as bacc
import concourse.bass as bass
import concourse.mybir as mybir
import concourse.tile as tile
from concourse.bass_utils import run_bass_kernel_spmd

import ml_dtypes

P = 128
BF16_NP = ml_dtypes.bfloat16
F32 = mybir.dt.float32
BF16 = mybir.dt.bfloat16
I16 = mybir.dt.int16
I32 = mybir.dt.int32
AF = mybir.ActivationFunctionType
ALU = mybir.AluOpType

GMAX = 40   # max slots per gather group
NBMAX = 12  # max blocks per gather group
CW = 512    # node-chunk width for dense matmuls


def _cdiv(a, b):
    return (a + b - 1) // b


# ----------------------------------------------------------------------------
# Host-side preprocessing: build per-core token tables from vidx/eidx.
# ----------------------------------------------------------------------------
class Prep:
    pass


def host_prep(vidx, eidx, N, E, C):
    """Build the static segment/gather structure shared by the SPMD program.

    Tokens for phase A (v->e): per core, entries sorted by eidx, grouped into
    NBE blocks of 128 edges; each block padded to a whole number of 128-token
    slots (count = max over cores, so the program is identical on all cores).
    Phase B (e->v) is the same with the roles of (node block, eidx) swapped.
    """
    p = Prep()
    NP = N // C
    NBE = _cdiv(E, P)
    NBV = _cdiv(NP, P)
    NPAD = NBV * P
    EPAD = NBE * P
    p.N, p.E, p.C, p.NP, p.NBE, p.NBV, p.NPAD, p.EPAD = N, E, C, NP, NBE, NBV, NPAD, EPAD
    assert NP < NPAD, "dummy row NP must live inside the padding"

    vidx = np.asarray(vidx).astype(np.int64)
    eidx = np.asarray(eidx).astype(np.int64)
    de = np.bincount(eidx, minlength=E).astype(np.float64)
    dv = np.bincount(vidx, minlength=N).astype(np.float64)
    de_inv = (1.0 / np.maximum(de, 1.0)).astype(np.float32)
    dv_inv = (1.0 / np.maximum(dv, 1.0)).astype(np.float32)
    core = vidx // NP

    # ---- phase A ----
    A_ev, A_lv = [], []
    cntA = np.zeros((C, NBE), np.int64)
    for c in range(C):
        m = core == c
        ev = eidx[m]
        lv = vidx[m] - c * NP
        o = np.argsort(ev, kind="stable")
        ev, lv = ev[o], lv[o]
        cntA[c] = np.bincount(ev // P, minlength=NBE)
        A_ev.append(ev)
        A_lv.append(lv)
    slotsA = np.maximum(1, _cdiv(cntA.max(0), P)).astype(np.int64)
    SA = int(slotsA.sum())
    TA = SA * P
    offA = np.zeros(NBE + 1, np.int64)
    np.cumsum(slotsA * P, out=offA[1:])

    idxA = np.full((C, TA), NP, np.int16)  # dummy -> zero row of T
    rA = np.zeros((C, TA), np.float32)
    for c in range(C):
        ev, lv = A_ev[c], A_lv[c]
        blk = ev // P
        starts = np.searchsorted(ev, np.arange(NBE) * P)
        tok = offA[blk] + (np.arange(len(ev)) - starts[blk])
        idxA[c, tok] = lv
        rA[c, tok] = ev - blk * P

    # ---- phase B ----
    B_ee, B_lv = [], []
    cntB = np.zeros((C, NBV), np.int64)
    for c in range(C):
        m = core == c
        lv = vidx[m] - c * NP
        ee = eidx[m]
        o = np.argsort(lv, kind="stable")
        lv, ee = lv[o], ee[o]
        cntB[c] = np.bincount(lv // P, minlength=NBV)
        B_ee.append(ee)
        B_lv.append(lv)
    slotsB = np.maximum(1, _cdiv(cntB.max(0), P)).astype(np.int64)
    SB = int(slotsB.sum())
    TB = SB * P
    offB = np.zeros(NBV + 1, np.int64)
    np.cumsum(slotsB * P, out=offB[1:])

    idxB = np.full((C, TB), E, np.int16)  # dummy -> zeroed row E of Ye
    rB = np.zeros((C, TB), np.float32)
    for c in range(C):
        ee, lv = B_ee[c], B_lv[c]
        blk = lv // P
        starts = np.searchsorted(lv, np.arange(NBV) * P)
        tok = offB[blk] + (np.arange(len(lv)) - starts[blk])
        idxB[c, tok] = ee
        rB[c, tok] = lv - blk * P

    # per-node dv_inv columns [C, 128, NBV]
    dvc = np.zeros((C, P, NBV), np.float32)
    for c in range(C):
        ids = c * NP + np.arange(NPAD)
        vals = np.where(ids < (c + 1) * NP, dv_inv[np.minimum(ids, N - 1)], 0.0)
        dvc[c] = vals.reshape(NBV, P).T

    # wrapped layouts for the device
    p.slotsA, p.slotsB, p.SA, p.SB, p.TA, p.TB = slotsA, slotsB, SA, SB, TA, TB
    p.offA, p.offB = offA, offB
    p.idxA_w = np.ascontiguousarray(np.tile(idxA.reshape(C, TA // 16, 16).transpose(0, 2, 1), (1, 8, 1)))
    p.rA_m = np.ascontiguousarray(rA.reshape(C, SA, P).transpose(0, 2, 1)).astype(BF16_NP)
    p.idxB_w = np.ascontiguousarray(np.tile(idxB.reshape(C, TB // 16, 16).transpose(0, 2, 1), (1, 8, 1)))
    p.rB_m = np.ascontiguousarray(rB.reshape(C, SB, P).transpose(0, 2, 1)).astype(BF16_NP)
    # de_inv per edge-block column [128, NBE] (same on all cores)
    dec = np.zeros(EPAD, np.float32)
    dec[:E] = de_inv
    p.dec = dec.reshape(NBE, P).T.copy()
    p.dvc = dvc
    assert int(slotsA.max()) <= GMAX and int(slotsB.max()) <= GMAX

    # gather groups: consecutive blocks, total slots <= gmax, <= maxb blocks
    def make_groups(slots, gmax, maxb):
        groups = []  # (block0, nblocks, slot0, gslots)
        b = 0
        nblk = len(slots)
        while b < nblk:
            s0 = int(slots[:b].sum())
            g = 0
            n = 0
            while b + n < nblk and n < maxb and g + slots[b + n] <= gmax:
                g += int(slots[b + n])
                n += 1
            assert n > 0, "single block exceeds gmax"
            groups.append((b, n, s0, g))
            b += n
        return groups

    p.groupsA = make_groups(slotsA, GMAX, NBMAX)
    p.groupsB = make_groups(slotsB, GMAX, NBMAX)
    return p


# ----------------------------------------------------------------------------
# Device program
# ----------------------------------------------------------------------------
def build_program(p, IN_DIM, H, OUT, L, fold=True, enable_asserts=False, stage=99,
                  async_gather=True):
    C, NP, NBE, NBV, NPAD, EPAD = p.C, p.NP, p.NBE, p.NBV, p.NPAD, p.EPAD
    KI = IN_DIM // P  # input-dim K tiles (3)
    KH = H // P  # hidden K tiles (2)
    assert IN_DIM % P == 0 and H % P == 0

    nc = bacc.Bacc(
        "TRN2",
        target_bir_lowering=False,
        debug=False,
        enable_asserts=enable_asserts,
        num_devices=C,
        num_swdge_queues=4,
    )

    # ---- I/O ----
    xT_d = nc.dram_tensor("xT", [IN_DIM, NPAD], BF16, kind="ExternalInput")
    encW_d = nc.dram_tensor("encW", [IN_DIM, H], BF16, kind="ExternalInput")
    encB_d = nc.dram_tensor("encB", [H], BF16, kind="ExternalInput")
    thW_d = nc.dram_tensor("thW", [L, H, H], BF16, kind="ExternalInput")
    thB_d = nc.dram_tensor("thB", [L, H], BF16, kind="ExternalInput")
    linW_d = nc.dram_tensor("linW", [H, OUT], BF16, kind="ExternalInput")
    linB_d = nc.dram_tensor("linB", [OUT], BF16, kind="ExternalInput")
    if not fold:
        lnG_d = nc.dram_tensor("lnG", [L, H], BF16, kind="ExternalInput")
        lnB_d = nc.dram_tensor("lnB", [L, H], BF16, kind="ExternalInput")
    idxA_d = nc.dram_tensor("idxA", [P, p.TA // 16], I16, kind="ExternalInput")
    rA_d = nc.dram_tensor("rA", [P, p.SA], BF16, kind="ExternalInput")
    idxB_d = nc.dram_tensor("idxB", [P, p.TB // 16], I16, kind="ExternalInput")
    rB_d = nc.dram_tensor("rB", [P, p.SB], BF16, kind="ExternalInput")
    dv_d = nc.dram_tensor("dvc", [P, NBV], F32, kind="ExternalInput")
    dec_d = nc.dram_tensor("dec", [P, NBE], F32, kind="ExternalInput")
    out_d = nc.dram_tensor("out", [NP, OUT], F32, kind="ExternalOutput")

    # ---- internals ----
    tt_d = nc.dram_tensor("tt_t", [NPAD, H], BF16)  # pre-conv activation (row-major)
    T_d = nc.dram_tensor("T_t", [NPAD, H], BF16)  # theta output; row NP stays zero
    h_d = nc.dram_tensor("h_t", [NPAD, H], BF16)  # residual state
    YeP_d = nc.dram_tensor("YeP", [EPAD, H], BF16)
    YeF_d = nc.dram_tensor("YeF", [EPAD, H], BF16, addr_space="Shared")

    last_rows = NP - (NBV - 1) * P  # valid rows in the final node block

    from contextlib import ExitStack
    with tile.TileContext(nc) as tc, ExitStack() as es:
        const = es.enter_context(tc.tile_pool(name="const", bufs=1))
        meta = es.enter_context(tc.tile_pool(name="meta", bufs=1))
        gpool = es.enter_context(tc.tile_pool(name="gpool", bufs=3))
        spool = es.enter_context(tc.tile_pool(name="spool", bufs=2))
        wpool = es.enter_context(tc.tile_pool(name="wpool", bufs=2))
        hpool = es.enter_context(tc.tile_pool(name="hpool", bufs=2))
        opool = es.enter_context(tc.tile_pool(name="opool", bufs=2))
        hnp = es.enter_context(tc.tile_pool(name="hnp", bufs=3))
        stat = es.enter_context(tc.tile_pool(name="stat", bufs=4))
        psA = es.enter_context(tc.tile_pool(name="psA", bufs=3, space="PSUM"))
        psT = es.enter_context(tc.tile_pool(name="psT", bufs=2, space="PSUM"))

        dma_sems = [nc.alloc_semaphore(f"swdge_dma_{q}") for q in range(4)]

        # ---- constants ----
        iota_f = const.tile([P, GMAX, P], BF16)
        with tc.tile_pool(name="iota_tmp", bufs=1) as itmp:
            iota_i = itmp.tile([P, 1, P], I32)
            nc.gpsimd.iota(iota_i[:, :, :], pattern=[[0, 1], [1, P]], base=0,
                           channel_multiplier=0)
            nc.vector.tensor_copy(iota_f[:, :, :],
                                  iota_i[:, 0:1, :].broadcast_to([P, GMAX, P]))
        ones1 = const.tile([1, P], BF16)
        nc.vector.memset(ones1[:, :], 1.0)
        epsc = const.tile([P, 1], F32)
        nc.vector.memset(epsc[:, :], 1e-5)

        # weights (rhs layout: [K-tile of 128, out-features])
        encW_t = []
        for k in range(KI):
            t = const.tile([P, H], BF16, tag=f"encW{k}")
            nc.sync.dma_start(t[:, :], encW_d[k * P:(k + 1) * P, :])
            encW_t.append(t)
        encB_t = const.tile([1, H], BF16)
        nc.sync.dma_start(encB_t[:, :], encB_d[None, :])
        thW_t = []
        for i in range(L):
            row = []
            for k in range(KH):
                t = const.tile([P, H], BF16, tag=f"thW{i}{k}")
                nc.sync.dma_start(t[:, :], thW_d[i, k * P:(k + 1) * P, :])
                row.append(t)
            thW_t.append(row)
        thB_t = []
        for i in range(L):
            t = const.tile([1, H], BF16, tag=f"thB{i}")
            nc.sync.dma_start(t[:, :], thB_d[i:i + 1, :])
            thB_t.append(t)
        linW_t = []
        for k in range(KH):
            t = const.tile([P, OUT], BF16, tag=f"linW{k}")
            nc.sync.dma_start(t[:, :], linW_d[k * P:(k + 1) * P, :])
            linW_t.append(t)
        linB_t = const.tile([1, OUT], BF16)
        nc.sync.dma_start(linB_t[:, :], linB_d[None, :])
        lnG_t, lnB_t = [], []
        if not fold:
            for i in range(L):
                g = const.tile([P, H], BF16, tag=f"lnG{i}")
                b = const.tile([P, H], BF16, tag=f"lnB{i}")
                nc.sync.dma_start(g[:, :], lnG_d[i:i + 1, :].partition_broadcast(P).squeeze(1))
                nc.sync.dma_start(b[:, :], lnB_d[i:i + 1, :].partition_broadcast(P).squeeze(1))
                lnG_t.append(g)
                lnB_t.append(b)

        # metadata
        idxA_t = meta.tile([P, p.TA // 16], I16)
        nc.sync.dma_start(idxA_t[:, :], idxA_d[:, :])
        rA_t = meta.tile([P, p.SA], BF16)
        nc.sync.dma_start(rA_t[:, :], rA_d[:, :])
        dec_t = meta.tile([P, NBE], F32)
        nc.sync.dma_start(dec_t[:, :], dec_d[:, :])
        idxB_t = meta.tile([P, p.TB // 16], I16)
        nc.sync.dma_start(idxB_t[:, :], idxB_d[:, :])
        rB_t = meta.tile([P, p.SB], BF16)
        nc.sync.dma_start(rB_t[:, :], rB_d[:, :])
        dv_t = meta.tile([P, NBV], F32)
        nc.sync.dma_start(dv_t[:, :], dv_d[:, :])

        # zero T_d's pad rows (incl. the dummy row NP); they are never rewritten
        zblk = const.tile([P, H], BF16)
        nc.vector.memset(zblk[:, :], 0.0)
        nc.sync.dma_start(T_d[NP:NPAD, :], zblk[:NPAD - NP, :])

        # row-major block views [128, nblocks, H]
        tt_view = tt_d.ap().rearrange("(c q) h -> q c h", q=P)
        T_view = T_d.ap().rearrange("(c q) h -> q c h", q=P)
        h_view = h_d.ap().rearrange("(c q) h -> q c h", q=P)
        YeP_view = YeP_d.ap().rearrange("(c q) h -> q c h", q=P)
        tt_tr = tt_d.ap().rearrange("m (k q) -> m k q", q=P)  # transpose-read view
        xT_view = xT_d.ap().rearrange("(k q) n -> q k n", q=P)

        qn = 0
        q_counts = [0, 0, 0, 0]

        def seg_gather(src_d, idx_t, s0, gs):
            """Issue one gather; returns (G tile, (sem, value)) where the
            consumer must wait_ge(sem, value) before reading G (async mode)."""
            nonlocal qn
            G = gpool.tile([P, GMAX, H], BF16, tag="G")
            tok0 = s0 * P
            kw = dict(prepare_only=True, sem=dma_sems[qn]) if async_gather else {}
            nc.gpsimd.dma_gather(
                out_ap=G[:, :gs, :],
                in_ap=src_d[:, :],
                idxs_ap=idx_t[:, tok0 // 16:(tok0 + gs * P) // 16],
                num_idxs=gs * P,
                num_idxs_reg=gs * P,
                elem_size=H,
                queue_num=qn,
                single_packet=False,
                **kw,
            )
            ready = None
            if async_gather:
                nc.gpsimd.trigger_dma(count=None, queue_num=qn)
                q_counts[qn] += 1
                ready = (dma_sems[qn], 16 * q_counts[qn])
            qn = (qn + 1) % 4
            return G, ready

        def seg_onehot(r_t, s0, gs):
            S = spool.tile([P, GMAX, P], BF16, tag="S")
            rb_ap = r_t[:, s0:s0 + gs].unsqueeze(2).broadcast_to([P, gs, P])
            nc.vector.tensor_tensor(S[:, :gs, :], iota_f[:, :gs, :], rb_ap,
                                    op=ALU.is_equal)
            return S

        # ------------------------------------------------------------------
        # Encoder: tt = x @ encW + encB (row-major, bf16)
        # ------------------------------------------------------------------
        for c0 in range(0, NPAD, CW):
            ncols = min(CW, NPAD - c0)
            nb = ncols // P
            cb0 = c0 // P
            xc = wpool.tile([P, KI, CW], BF16, tag="xc")
            nc.sync.dma_start(xc[:, :, :ncols], xT_view[:, :, c0:c0 + ncols])
            ttO = opool.tile([P, CW // P, H], BF16, tag="TO")
            for b in range(nb):
                ps = psA.tile([P, H], F32, tag="ps")
                for k in range(KI):
                    nc.tensor.matmul(ps[:, :], lhsT=xc[:, k, b * P:(b + 1) * P],
                                     rhs=encW_t[k][:, :], start=(k == 0), stop=False)
                nc.tensor.matmul(ps[:, :], lhsT=ones1[:1, :], rhs=encB_t[:1, :],
                                 start=False, stop=True)
                nc.scalar.activation(ttO[:, b, :], ps[:, :], AF.Copy)
            nc.scalar.dma_start(tt_view[:, cb0:cb0 + nb, :], ttO[:, :nb, :])

        # ------------------------------------------------------------------
        # Conv layers
        # ------------------------------------------------------------------
        for li in range(L if stage >= 2 else 0):
            # ---- T = tt @ thetaW[li] + thB[li] (row-major bf16) ----
            for c0 in range(0, NPAD, CW):
                ncols = min(CW, NPAD - c0)
                nb = ncols // P
                cb0 = c0 // P
                ttT = wpool.tile([P, KH, CW], BF16, tag="ttT")
                for k in range(KH):
                    nc.sync.dma_start(ttT[:, k, :ncols], tt_tr[c0:c0 + ncols, k, :],
                                      transpose=True)
                TO = opool.tile([P, CW // P, H], BF16, tag="TO")
                for b in range(nb):
                    ps = psA.tile([P, H], F32, tag="ps")
                    for k in range(KH):
                        nc.tensor.matmul(ps[:, :], lhsT=ttT[:, k, b * P:(b + 1) * P],
                                         rhs=thW_t[li][k][:, :], start=(k == 0), stop=False)
                    nc.tensor.matmul(ps[:, :], lhsT=ones1[:1, :], rhs=thB_t[li][:1, :],
                                     start=False, stop=True)
                    nc.scalar.activation(TO[:, b, :], ps[:, :], AF.Copy)
                if cb0 + nb == NBV:  # chunk contains the final (partial) block
                    if nb > 1:
                        nc.sync.dma_start(T_view[:, cb0:cb0 + nb - 1, :], TO[:, :nb - 1, :])
                    nc.sync.dma_start(T_d[(NBV - 1) * P:NP, :], TO[:last_rows, nb - 1, :])
                else:
                    nc.sync.dma_start(T_view[:, cb0:cb0 + nb, :], TO[:, :nb, :])

            # ---- Phase A: partial Ye ----
            for (b0, nbk, s0, gs) in (p.groupsA if stage >= 3 else []):
                G, ready = seg_gather(T_d, idxA_t, s0, gs)
                S = seg_onehot(rA_t, s0, gs)
                yeO = opool.tile([P, NBMAX, H], BF16, tag="yeO")
                if ready is not None:
                    nc.tensor.wait_ge(ready[0], ready[1])
                ls = 0
                for i in range(nbk):
                    eb = b0 + i
                    sb = int(p.slotsA[eb])
                    ps = psA.tile([P, H], F32, tag="ps")
                    for s in range(sb):
                        nc.tensor.matmul(ps[:, :], lhsT=S[:, ls + s, :], rhs=G[:, ls + s, :],
                                         start=(s == 0), stop=(s == sb - 1))
                    nc.scalar.activation(yeO[:, i, :], ps[:, :], AF.Copy,
                                         scale=dec_t[:, eb:eb + 1])
                    ls += sb
                nc.sync.dma_start(YeP_view[:, b0:b0 + nbk, :], yeO[:, :nbk, :])

            # ---- AllReduce hyperedge partials ----
            if stage < 4:
                continue
            nc.gpsimd.collective_compute(
                "AllReduce",
                ALU.add,
                replica_groups=[list(range(C))],
                ins=[YeP_d.ap()[:EPAD, :]],
                outs=[YeF_d.ap()[:EPAD, :]],
            )

            # ---- Phase B: conv output + residual + LN tail ----
            lnxt = li + 1 if li + 1 < L else 0
            for (b0, nbk, s0, gs) in (p.groupsB if stage >= 5 else []):
                G, ready = seg_gather(YeF_d, idxB_t, s0, gs)
                S = seg_onehot(rB_t, s0, gs)
                if ready is not None:
                    nc.tensor.wait_ge(ready[0], ready[1])
                if li > 0:
                    hP = hpool.tile([P, NBMAX, H], BF16, tag="hP")
                    nc.scalar.dma_start(hP[:, :nbk, :], h_view[:, b0:b0 + nbk, :])
                hO = opool.tile([P, NBMAX, H], BF16, tag="hO")
                ttO = opool.tile([P, NBMAX, H], BF16, tag="ttO")
                ls = 0
                for i in range(nbk):
                    vb = b0 + i
                    sb = int(p.slotsB[vb])
                    ps = psA.tile([P, H], F32, tag="ps")
                    for s in range(sb):
                        nc.tensor.matmul(ps[:, :], lhsT=S[:, ls + s, :], rhs=G[:, ls + s, :],
                                         start=(s == 0), stop=(s == sb - 1))
                    ls += sb
                    # relu(dv * x) (== dv * relu(x), dv >= 0)
                    hn = hnp.tile([P, H], F32, tag="hn")
                    nc.scalar.activation(hn[:, :], ps[:, :], AF.Relu,
                                         scale=dv_t[:, vb:vb + 1])
                    if li > 0:
                        nc.vector.tensor_tensor(hn[:, :], hn[:, :], hP[:, i, :],
                                                op=ALU.add)
                    nc.vector.tensor_copy(hO[:, i, :], hn[:, :])

                    # tail: tt = relu(LN_lnxt(hn)) (gain/bias folded into weights)
                    if stage < 6:
                        continue
                    st6 = stat.tile([P, 6], F32, tag="st6")
                    nc.vector.bn_stats(st6[:, :], hn[:, :])
                    mv = stat.tile([P, 2], F32, tag="mv")
                    nc.vector.bn_aggr(mv[:, :], st6[:, :])
                    rstd = stat.tile([P, 1], F32, tag="rstd")
                    nc.scalar.activation(rstd[:, :], mv[:, 1:2], AF.Sqrt,
                                         bias=epsc[:, :], scale=1.0)
                    rinv = stat.tile([P, 1], F32, tag="rinv")
                    nc.vector.reciprocal(rinv[:, :], rstd[:, :])
                    nmr = stat.tile([P, 1], F32, tag="nmr")
                    nc.vector.tensor_scalar(nmr[:, :], mv[:, 0:1], rinv[:, :], -1.0,
                                            op0=ALU.mult, op1=ALU.mult)
                    if fold:
                        nc.scalar.activation(ttO[:, i, :], hn[:, :], AF.Relu,
                                             bias=nmr[:, :], scale=rinv[:, :])
                    else:
                        tn = hnp.tile([P, H], F32, tag="tn")
                        nc.vector.tensor_scalar(tn[:, :], hn[:, :], mv[:, 0:1],
                                                rinv[:, :], op0=ALU.subtract,
                                                op1=ALU.mult)
                        nc.vector.tensor_tensor(tn[:, :], tn[:, :], lnG_t[lnxt][:, :],
                                                op=ALU.mult)
                        nc.vector.tensor_tensor(tn[:, :], tn[:, :], lnB_t[lnxt][:, :],
                                                op=ALU.add)
                        nc.scalar.activation(ttO[:, i, :], tn[:, :], AF.Relu)
                nc.scalar.dma_start(h_view[:, b0:b0 + nbk, :], hO[:, :nbk, :])
                if stage >= 6:
                    nc.scalar.dma_start(tt_view[:, b0:b0 + nbk, :], ttO[:, :nbk, :])

        # ------------------------------------------------------------------
        # Final: out = tt @ linW + linB  (tt == relu(LN_0(h)), gain folded)
        # ------------------------------------------------------------------
        for c0 in range(0, NPAD, CW):
            ncols = min(CW, NPAD - c0)
            nb = ncols // P
            cb0 = c0 // P
            ttT = wpool.tile([P, KH, CW], BF16, tag="ttT")
            for k in range(KH):
                nc.sync.dma_start(ttT[:, k, :ncols], tt_tr[c0:c0 + ncols, k, :],
                                  transpose=True)
            oO = opool.tile([P, CW // P, OUT], F32, tag="oO")
            for b in range(nb):
                ps = psT.tile([P, OUT], F32, tag="psT")
                for k in range(KH):
                    nc.tensor.matmul(ps[:, :], lhsT=ttT[:, k, b * P:(b + 1) * P],
                                     rhs=linW_t[k][:, :], start=(k == 0), stop=False)
                nc.tensor.matmul(ps[:, :], lhsT=ones1[:1, :], rhs=linB_t[:1, :],
                                 start=False, stop=True)
                nc.scalar.activation(oO[:, b, :], ps[:, :], AF.Copy)
            if cb0 + nb == NBV:
                out_full = out_d.ap()[:(NBV - 1) * P, :].rearrange("(c q) o -> q c o", q=P)
                if nb > 1:
                    nc.sync.dma_start(out_full[:, cb0:cb0 + nb - 1, :], oO[:, :nb - 1, :])
                nc.sync.dma_start(out_d[(NBV - 1) * P:NP, :], oO[:last_rows, nb - 1, :])
            else:
                out_full = out_d.ap()[:(NBV - 1) * P, :].rearrange("(c q) o -> q c o", q=P)
                nc.sync.dma_start(out_full[:, cb0:cb0 + nb, :], oO[:, :nb, :])

    nc.compile()
    return nc


# ----------------------------------------------------------------------------
# Full pipeline: prep + build + run
# ----------------------------------------------------------------------------
def run_full(x, vidx, eidx, encW, encB, thetaW, thetaB, lnG, lnB, linW, linB,
             N, E, C, trace=False, nc_cache=None, stage=99, async_gather=True,
             **runkw):
    IN_DIM = x.shape[1]
    H = encW.shape[1]
    OUT = linW.shape[1]
    L = thetaW.shape[0]

    p = host_prep(np.asarray(vidx), np.asarray(eidx), N, E, C)

    lnG = np.asarray(lnG, np.float32)
    lnB = np.asarray(lnB, np.float32)
    thetaW = np.asarray(thetaW, np.float32)
    linW = np.asarray(linW, np.float32)
    fold = bool((lnB == 0).all() and (lnG > 0).all())
    thW_eff = thetaW.copy()
    linW_eff = linW
    if fold:
        for i in range(1, L):
            thW_eff[i] = lnG[i][:, None] * thetaW[i]
        linW_eff = lnG[0][:, None] * linW

    nc = nc_cache if nc_cache is not None else build_program(
        p, IN_DIM, H, OUT, L, fold=fold, stage=stage, async_gather=async_gather)

    x = np.asarray(x, np.float32)
    NP, NPAD = p.NP, p.NPAD
    in_maps = []
    for c in range(C):
        xs = x[c * NP:(c + 1) * NP]
        xT = np.zeros((IN_DIM, NPAD), BF16_NP)
        xT[:, :NP] = xs.T.astype(BF16_NP)
        m = dict(
            xT=xT,
            encW=np.asarray(encW, BF16_NP),
            encB=np.asarray(encB, BF16_NP),
            thW=thW_eff.astype(BF16_NP),
            thB=np.asarray(thetaB, BF16_NP),
            linW=linW_eff.astype(BF16_NP),
            linB=np.asarray(linB, BF16_NP),
            idxA=p.idxA_w[c],
            rA=p.rA_m[c],
            idxB=p.idxB_w[c],
            rB=p.rB_m[c],
            dvc=p.dvc[c],
            dec=p.dec,
        )
        if not fold:
            m["lnG"] = lnG.astype(BF16_NP)
            m["lnB"] = lnB.astype(BF16_NP)
        in_maps.append(m)

    res = run_bass_kernel_spmd(nc, in_maps, core_ids=list(range(C)), trace=trace, **runkw)
    out = np.concatenate([res.results[c]["out"] for c in range(C)], axis=0)
    return out, res, nc, p


# hardcoded problem configuration (nn_DeeperHNN_88295937671288)
_N, _E, _NNZ = 100000, 20000, 800000
_C = 8

_nc_cache = None


import os
_ASYNC = os.environ.get("KERNEL_ASYNC_GATHER", "1") == "1"
_STAGE = int(os.environ.get("KERNEL_STAGE", "99"))


def kernel(x, vidx, eidx, encW, encB, thetaW, thetaB, lnG, lnB, linW, linB):
    global _nc_cache
    out, res, nc, p = run_full(
        x, vidx, eidx, encW, encB, thetaW, thetaB, lnG, lnB, linW, linB,
        N=_N, E=_E, C=_C, nc_cache=None, stage=_STAGE, async_gather=_ASYNC,
    )
    _nc_cache = nc
    return out.astype(np.float32)
